# revision 53
# baseline (speedup 1.0000x reference)
"""BERT self-attention Bass/Tile kernel for 8 Trainium2 NeuronCores.

Problem: hidden [2, 2048, 768], 12 heads x 64 dim, additive mask [2,1,1,2048].
Sharding: batch x head-group. Core c handles batch b = c // 4 and global heads
3*(c%4) .. 3*(c%4)+2 (columns 192*(c%4) .. +192 of Wq/Wk/Wv).  Each core
computes its 3 heads' full attention locally; outputs are concatenated on the
host (no cross-device communication).

Per-core pipeline (all in one TileContext):
  X [2048,768] --cast fp16, PE transpose--> X_T [768, 2048]
  Q_T/K_T/V_T = W.T @ X_T   (fp16 matmuls; pair of heads packed M=128 + solo)
  scores_T[k,q] = K_T.T @ Q_T  (K=64 contraction; two row-tiled streams
                                interleaved per k-chunk so the PE overlaps them)
  probs = exp(scores/8) via ScalarE (PSUM -> SBUF, fp16)
  V[k] layout [V_h0|e|V_h1|e|V_h2|e] where e = exp(mask_k) column
  ctx_aug[q, 65] = probs_T.T @ V_aug  (col 64 = softmax denominator)
  out[q, d] = ctx[:, :64] * (1 / ctx[:, 64])   -> DMA to DRAM

The additive mask is folded into V: exp(s + m_k) = exp(s) * exp(m_k), so both
the numerator and the denominator column of V_aug are pre-scaled by exp(m_k).
When the mask is all zeros (the common case) that scale is skipped and the
denominator column is just memset to 1.
"""

import os

import numpy as np

import concourse.bass as bass
import concourse.bass_utils as _bass_utils
import concourse.tile as tile
from concourse import bacc, mybir
from concourse.bass_utils import run_bass_kernel_spmd
from concourse.masks import make_identity

# walrus is invoked with --enable-ldw-opt=false by default; the LDWEIGHTS
# double-buffer optimization hides weight-load time behind the previous
# matmul, which matters a lot for this kernel's many small matmuls.
_orig_run_command = _bass_utils.run_command


def _run_command_ldw(cmd, *a, **kw):
    cmd = [c.replace("--enable-ldw-opt=false", "--enable-ldw-opt=true")
           if isinstance(c, str) else c for c in cmd]
    return _orig_run_command(cmd, *a, **kw)


# NOTE: tried and reverted — walrus rejects explicit InstLdweights (which bass
# emits for 2-byte matmuls) when the opt is on: "InstLdweights is not
# compatible with LDW optimization".
if os.environ.get("ATTN_LDW_OPT", "0") == "1":
    _bass_utils.run_command = _run_command_ldw

F32 = mybir.dt.float32
F16 = mybir.dt.float16
EXP = mybir.ActivationFunctionType.Exp

S = 2048           # sequence length
DM = 768           # model dim
DH = 64            # head dim
NHL = 3            # local heads per core
FC = DM // 128     # 6 f-chunks (contraction for projections)
KC = S // 128      # 16 k-chunks
QB = 512           # q block width for score matmuls (one mm per k-chunk)
NQB = S // QB      # 4 q blocks
# k-chunk groups per exp op: [128, 2*512] PSUM tile = 2 banks
GROUPS = [(2 * i, 2) for i in range(8)]


def _build_kernel(zero_mask: bool) -> bass.Bass:
    nc = bacc.Bacc()

    x_d = nc.declare_dram_parameter("x", [S, DM], F32, isOutput=False)
    wq_d = nc.declare_dram_parameter("wq", [DM, 192], F32, isOutput=False)
    wk_d = nc.declare_dram_parameter("wk", [DM, 192], F32, isOutput=False)
    wv_d = nc.declare_dram_parameter("wv", [DM, 192], F32, isOutput=False)
    bq_d = nc.declare_dram_parameter("bq", [192], F32, isOutput=False)
    bk_d = nc.declare_dram_parameter("bk", [192], F32, isOutput=False)
    bv_d = nc.declare_dram_parameter("bv", [192], F32, isOutput=False)
    m_d = nc.declare_dram_parameter("mask", [S], F32, isOutput=False)
    out_d = nc.declare_dram_parameter("out", [S, 192], F32, isOutput=True)

    with tile.TileContext(nc) as tc:
        _attention(tc, x_d, (wq_d, wk_d, wv_d), (bq_d, bk_d, bv_d), m_d, out_d,
                   zero_mask)
    nc.compile()
    return nc


def _attention(tc, x_d, w_ds, b_ds, m_d, out_d, zero_mask):
    nc = tc.nc

    const = tc.alloc_tile_pool(name="const", bufs=1)
    xpool = tc.alloc_tile_pool(name="xpool", bufs=5)
    persist = tc.alloc_tile_pool(name="persist", bufs=1)
    probs_pool = tc.alloc_tile_pool(name="probs", bufs=50)
    small = tc.alloc_tile_pool(name="small", bufs=4)
    outp = tc.alloc_tile_pool(name="outp", bufs=1)
    ps = tc.alloc_tile_pool(name="ps", bufs=2, space="PSUM")

    # --- constants: mask, identity, weights, biases -------------------------
    # All small loads go through gpsimd (SWDGE) so the SP sequencer's serial
    # ~0.65us-per-DMA issue cost is spent exclusively on the large X loads.
    mask_t = const.tile([128, KC], F32)  # mask[128*i + p] at [p, i]
    nc.gpsimd.dma_start(out=mask_t, in_=m_d[:].rearrange("(i p) -> p i", p=128))
    expm = const.tile([128, KC], F32)    # exp(mask), per k position
    nc.scalar.activation(expm, mask_t, EXP)  # early: also triggers table load

    ident16 = const.tile([128, 128], F16)
    make_identity(nc, ident16)

    w16 = {}   # (t, f) -> [128, 192] fp16
    for t in range(3):
        for f in range(FC):
            w32 = small.tile([128, 192], F32, name=f"w32_{t}_{f}", tag="w32",
                             bufs=9)
            nc.gpsimd.dma_start(out=w32, in_=w_ds[t][128 * f:128 * (f + 1), :])
            wt = const.tile([128, 192], F16, name=f"w16_{t}_{f}")
            nc.vector.tensor_copy(out=wt, in_=w32)
            w16[(t, f)] = wt

    bias_pair = []
    bias_solo = []
    for t in range(3):
        bp = const.tile([128, 1], F32, name=f"bias_pair_{t}")
        nc.gpsimd.dma_start(out=bp, in_=b_ds[t][0:128].rearrange("(p o) -> p o", o=1))
        bias_pair.append(bp)
        bs = const.tile([64, 1], F32, name=f"bias_solo_{t}")
        nc.gpsimd.dma_start(out=bs, in_=b_ds[t][128:192].rearrange("(p o) -> p o", o=1))
        bias_solo.append(bs)

    # --- persistent projection outputs --------------------------------------
    # QT2/KT2: [128, 2048] fp16, rows 0:64 = head0, 64:128 = head1
    # QTs/KTs: [128, 2048] fp16, head2 duplicated into both partition halves
    XT = [persist.tile([128, S], F16, name=f"XT_{f}") for f in range(FC)]
    QT2 = persist.tile([128, S], F16)
    KT2 = persist.tile([128, S], F16)
    QTs = persist.tile([128, S], F16)
    KTs = persist.tile([128, S], F16)
    VT2 = persist.tile([128, S], F16)
    VTs = persist.tile([64, S], F16)
    # V[kc] layout: [V_h0(64) | e | V_h1(64) | e | V_h2(64) | e], e = exp(m_k)
    V = [persist.tile([128, 195], F16, name=f"V_{kc}") for kc in range(KC)]

    out_tiles = [outp.tile([128, 192], F32, name=f"o_{u}") for u in range(16)]
    out_written = [0] * 16

    def load_x_block(m):
        """DMA 4 q-tiles of X, cast fp16 on gpsimd, PE-transpose into XT."""
        x16s = []
        for j in range(4):
            qt = 4 * m + j
            xt32 = xpool.tile([128, DM], F32, name=f"x_{qt}", tag="x")
            nc.sync.dma_start(out=xt32, in_=x_d[128 * qt:128 * (qt + 1), :])
            x16 = xpool.tile([128, DM], F16, name=f"x16_{qt}", tag="x16")
            nc.vector.tensor_copy(out=x16, in_=xt32)
            x16s.append(x16)
        for f in range(FC):
            tp = ps.tile([128, 512], F16, name=f"xt_ps_{m}_{f}", tag="sm", bufs=4)
            for j in range(4):
                nc.tensor.transpose(
                    tp[:, 128 * j:128 * (j + 1)],
                    x16s[j][:, 128 * f:128 * (f + 1)],
                    ident16,
                )
            nc.vector.tensor_copy(out=XT[f][:, 512 * m:512 * (m + 1)], in_=tp)

    def proj_pair(t, dst_pair, m):
        cols = slice(512 * m, 512 * (m + 1))
        pp = ps.tile([128, 512], F32, name=f"proj_{t}_{m}_p", tag="sm", bufs=4)
        for f in range(FC):
            nc.tensor.matmul(pp, w16[(t, f)][:, 0:128], XT[f][:, cols],
                             start=(f == 0), stop=(f == FC - 1))
        nc.vector.tensor_scalar_add(out=dst_pair[:, cols], in0=pp,
                                    scalar1=bias_pair[t])

    def proj_solo2(ta, tb, dst_a, dst_b, m):
        """Two M=64 head-2 projections col-tiled into one PSUM bank: ta's
        output at partitions 0:64, tb's at 64:128, sharing the streamed X_T.
        Only ta's first matmul carries start=True (it clears the whole bank's
        has_written bits); tb's first is pinned after it and overwrites its
        cleared region."""
        cols = slice(512 * m, 512 * (m + 1))
        sp = ps.tile([128, 512], F32, name=f"proj_s_{m}", tag="sm", bufs=4)
        prev = None
        for f in range(FC):
            ma = nc.tensor.matmul(sp[0:64], w16[(ta, f)][:, 128:192],
                                  XT[f][:, cols],
                                  start=(f == 0), stop=(f == FC - 1))
            if prev is not None:
                tile.add_dep_helper(ma.ins, prev.ins, sync=False,
                                    reason="solo col-pair order")
            mb = nc.tensor.matmul(sp[64:128], w16[(tb, f)][:, 128:192],
                                  XT[f][:, cols],
                                  start=False, stop=(f == FC - 1))
            tile.add_dep_helper(mb.ins, ma.ins, sync=False,
                                reason="solo col-pair order")
            prev = mb
        nc.vector.tensor_scalar_add(out=dst_a[0:64, cols], in0=sp[0:64],
                                    scalar1=bias_solo[ta])
        nc.vector.tensor_scalar_add(out=dst_b[64:128, cols], in0=sp[64:128],
                                    scalar1=bias_solo[tb])
        # duplicate head2 into the other partition half for row tiling
        nc.sync.dma_start(out=dst_a[64:128, cols], in_=dst_a[0:64, cols])
        nc.sync.dma_start(out=dst_b[0:64, cols], in_=dst_b[64:128, cols])

    def proj_solo1(t, dst, m):
        cols = slice(512 * m, 512 * (m + 1))
        sp = ps.tile([128, 512], F32, name=f"proj_v_{m}", tag="sm", bufs=4)
        for f in range(FC):
            nc.tensor.matmul(sp[0:64], w16[(t, f)][:, 128:192], XT[f][:, cols],
                             start=(f == 0), stop=(f == FC - 1))
        nc.vector.tensor_scalar_add(out=dst[0:64, cols], in0=sp[0:64],
                                    scalar1=bias_solo[t])

    def build_v(kc):
        """Transpose V_T chunk back to [k, d], scale by exp(mask), add e col."""
        ks = slice(128 * kc, 128 * (kc + 1))
        vp = ps.tile([128, 192], F16, name=f"v_ps_{kc}", tag="sm", bufs=4)
        nc.tensor.transpose(vp[:, 0:128], VT2[:, ks], ident16)
        nc.tensor.transpose(vp[:, 128:192], VTs[:, ks], ident16[0:64, 0:64])
        ecol = bass.AP(tensor=V[kc].tensor, offset=V[kc].offset + 64,
                       ap=[V[kc].ap[0], [65, 3]])
        if zero_mask:
            for h in range(NHL):
                nc.vector.tensor_copy(
                    out=V[kc][:, 65 * h:65 * h + 64],
                    in_=vp[:, 64 * h:64 * h + 64])
            nc.gpsimd.memset(ecol, 1.0)
        else:
            sc = expm[:, kc:kc + 1]
            for h in range(NHL):
                nc.vector.tensor_scalar_mul(
                    out=V[kc][:, 65 * h:65 * h + 64],
                    in0=vp[:, 64 * h:64 * h + 64], scalar1=sc)
            esrc = bass.AP(tensor=expm.tensor, offset=expm.offset + kc,
                           ap=[expm.ap[0], [0, 3]])
            nc.vector.tensor_copy(out=ecol, in_=esrc)

    def scores_group(streams, g):
        """Row-tile-interleaved score matmuls + exp for two streams.

        streams: [(head, J, prow), (head, J, prow)] with prow 0 and 64.
        J is a 512-wide q block; one N=512 matmul per k-chunk per stream.
        Returns the two probs tiles."""
        k0, kn = GROUPS[g]
        scs = []
        for (head, J, prow) in streams:
            scs.append(ps.tile([128, QB * kn], F32,
                               name=f"sc_{head}_{J}_{g}", tag="sc", bufs=2))
        # N=256 half-matmuls, alternating the two streams' row groups: the PE
        # runs rows 0-63 and 64-127 concurrently (measured pair start skew
        # ~4ns), which doubles effective throughput for this K=64 contraction.
        prev_mm = None
        for j in range(kn):
            kc = k0 + j
            for h in range(2):
                for i, (head, J, prow) in enumerate(streams):
                    KT = KT2 if head < 2 else KTs
                    QT = QT2 if head < 2 else QTs
                    mm = nc.tensor.matmul(
                        scs[i][:, QB * j + 256 * h:QB * j + 256 * (h + 1)],
                        KT[prow:prow + 64, 128 * kc:128 * (kc + 1)],
                        QT[prow:prow + 64,
                           QB * J + 256 * h:QB * J + 256 * (h + 1)],
                        start=True, stop=True)
                    if i == 1 and prev_mm is not None:
                        tile.add_dep_helper(mm.ins, prev_mm.ins, sync=False,
                                            reason="score pair adjacency")
                    prev_mm = mm if i == 0 else None
        pts = []
        for i, (head, J, prow) in enumerate(streams):
            pt = probs_pool.tile([128, QB * kn], F16,
                                 name=f"pb_{head}_{J}_{g}", tag="probs")
            nc.scalar.activation(pt, scs[i], EXP, scale=0.125)
            pts.append(pt)
        return pts

    def ctx_chain(head, J, probs, s):
        """One q-sub-chunk's ctx accumulation + normalize + out.

        One PSUM tile (= one bank) per accumulation chain: start=True clears
        has_written for the whole bank, so chains must not share a bank."""
        cx = ps.tile([128, 65], F32, name=f"cx_{head}_{J}_{s}", tag="sm", bufs=4)
        for g, (k0, kn) in enumerate(GROUPS):
            for j in range(kn):
                kc = k0 + j
                nc.tensor.matmul(
                    cx,
                    probs[g][:, QB * j + 128 * s:QB * j + 128 * (s + 1)],
                    V[kc][:, 65 * head:65 * head + 65],
                    start=(kc == 0), stop=(kc == KC - 1))
        r = small.tile([128, 1], F32, name=f"r_{head}_{J}_{s}", tag="recip")
        nc.vector.reciprocal(r, cx[:, 64:65])
        u = 4 * J + s
        nc.vector.tensor_scalar_mul(
            out=out_tiles[u][:, 64 * head:64 * (head + 1)],
            in0=cx[:, 0:64], scalar1=r)
        out_written[u] += 1
        if out_written[u] == NHL:
            nc.sync.dma_start(out=out_d[128 * u:128 * (u + 1), :],
                              in_=out_tiles[u])

    # --- emission ------------------------------------------------------------
    # Score groups for ALL steps are spread across the projection m-blocks as
    # soon as their K/Q column blocks exist, so ScalarE's exp work overlaps the
    # whole projection phase.  A cap on un-consumed probs tiles bounds SBUF.
    all_steps = [
        ([(0, 0, 0), (1, 0, 64)], 0),
        ([(0, 1, 0), (1, 1, 64)], 1),
        ([(2, 0, 0), (2, 1, 64)], 1),
        ([(0, 2, 0), (1, 2, 64)], 2),
        ([(0, 3, 0), (1, 3, 64)], 3),
        ([(2, 2, 0), (2, 3, 64)], 3),
    ]
    units = [(si, g) for si in range(len(all_steps)) for g in range(len(GROUPS))]
    emitted = set()
    step_probs = {si: [[None] * len(GROUPS) for _ in range(2)]
                  for si in range(len(all_steps))}
    state = {"inflight": 0}
    CAP = 21  # max un-consumed score groups (2 probs tiles each)

    def emit_unit(si, g):
        streams, _ = all_steps[si]
        pts = scores_group(streams, g)
        step_probs[si][0][g] = pts[0]
        step_probs[si][1][g] = pts[1]
        emitted.add((si, g))
        state["inflight"] += 1

    def try_emit(q_m, k_m, budget, pair_only=False):
        for (si, g) in units:
            if budget <= 0 or state["inflight"] >= CAP:
                return
            if (si, g) in emitted:
                continue
            if pair_only and all_steps[si][0][0][0] == 2:
                continue
            k0, kn = GROUPS[g]
            if all_steps[si][1] <= q_m and (k0 + kn - 1) // 4 <= k_m:
                emit_unit(si, g)
                budget -= 1

    for m in range(4):
        load_x_block(m)
        proj_pair(1, KT2, m)   # K first: scores need all of K
        try_emit(m - 1, m - 1, 3)
        proj_pair(0, QT2, m)
        try_emit(m, m, 3, pair_only=True)
        proj_solo2(0, 1, QTs, KTs, m)
        try_emit(m, m, 3)
        proj_pair(2, VT2, m)
        try_emit(m, m, 3)
        proj_solo1(2, VTs, m)
        for kc in range(4 * m, 4 * m + 4):
            build_v(kc)
        try_emit(m, m, 3)

    # Steady state: remaining score groups interleaved with ctx chains of
    # completed steps, so the PE's ctx work overlaps ScalarE's exp work.
    pending = []
    for si, (streams, _) in enumerate(all_steps):
        for g in range(len(GROUPS)):
            if (si, g) not in emitted:
                if pending:
                    ctx_chain(*pending.pop(0))
                emit_unit(si, g)
        for s in range(4):
            for i in range(2):
                pending.append((streams[i][0], streams[i][1],
                                step_probs[si][i], s))
        state["inflight"] -= len(GROUPS)
    while pending:
        ctx_chain(*pending.pop(0))

    for p in (ps, outp, small, probs_pool, persist, xpool, const):
        p.release()


_NC_CACHE = {}


def _get_nc(zero_mask: bool):
    if zero_mask not in _NC_CACHE:
        _NC_CACHE[zero_mask] = _build_kernel(zero_mask)
    return _NC_CACHE[zero_mask]


def kernel(hidden_states, attention_mask, Wq, bq, Wk, bk, Wv, bv, **run_kw):
    hidden_states = np.asarray(hidden_states, dtype=np.float32)
    attention_mask = np.asarray(attention_mask, dtype=np.float32)
    Wq, Wk, Wv = (np.asarray(a, dtype=np.float32) for a in (Wq, Wk, Wv))
    bq, bk, bv = (np.asarray(a, dtype=np.float32) for a in (bq, bk, bv))

    zero_mask = bool(np.all(attention_mask == 0.0))
    nc = _get_nc(zero_mask)
    in_maps = []
    for c in range(8):
        b, g = c // 4, c % 4
        cols = slice(192 * g, 192 * (g + 1))
        in_maps.append({
            "x": np.ascontiguousarray(hidden_states[b]),
            "wq": np.ascontiguousarray(Wq[:, cols]),
            "wk": np.ascontiguousarray(Wk[:, cols]),
            "wv": np.ascontiguousarray(Wv[:, cols]),
            "bq": np.ascontiguousarray(bq[cols]),
            "bk": np.ascontiguousarray(bk[cols]),
            "bv": np.ascontiguousarray(bv[cols]),
            "mask": np.ascontiguousarray(
                np.broadcast_to(attention_mask[b, 0, 0], (S,))),
        })
    res = run_bass_kernel_spmd(nc, in_maps, list(range(8)), **run_kw)
    out = np.empty((2, S, DM), dtype=np.float32)
    for c in range(8):
        b, g = c // 4, c % 4
        out[b, :, 192 * g:192 * (g + 1)] = res.results[c]["out"]
    if run_kw:
        return out, res
    return out


# revision 55
# speedup vs baseline: 1.0057x; 1.0057x over previous
"""BERT self-attention Bass/Tile kernel for 8 Trainium2 NeuronCores.

Problem: hidden [2, 2048, 768], 12 heads x 64 dim, additive mask [2,1,1,2048].
Sharding: batch x head-group. Core c handles batch b = c // 4 and global heads
3*(c%4) .. 3*(c%4)+2 (columns 192*(c%4) .. +192 of Wq/Wk/Wv).  Each core
computes its 3 heads' full attention locally; outputs are concatenated on the
host (no cross-device communication).

Per-core pipeline (all in one TileContext):
  X [2048,768] --cast fp16, PE transpose--> X_T [768, 2048]
  Q_T/K_T/V_T = W.T @ X_T   (fp16 matmuls; pair of heads packed M=128 + solo)
  scores_T[k,q] = K_T.T @ Q_T  (K=64 contraction; two row-tiled streams
                                interleaved per k-chunk so the PE overlaps them)
  probs = exp(scores/8) via ScalarE (PSUM -> SBUF, fp16)
  V[k] layout [V_h0|e|V_h1|e|V_h2|e] where e = exp(mask_k) column
  ctx_aug[q, 65] = probs_T.T @ V_aug  (col 64 = softmax denominator)
  out[q, d] = ctx[:, :64] * (1 / ctx[:, 64])   -> DMA to DRAM

The additive mask is folded into V: exp(s + m_k) = exp(s) * exp(m_k), so both
the numerator and the denominator column of V_aug are pre-scaled by exp(m_k).
When the mask is all zeros (the common case) that scale is skipped and the
denominator column is just memset to 1.
"""

import os

import numpy as np

import concourse.bass as bass
import concourse.bass_utils as _bass_utils
import concourse.tile as tile
from concourse import bacc, mybir
from concourse.bass_utils import run_bass_kernel_spmd
from concourse.masks import make_identity

# walrus is invoked with --enable-ldw-opt=false by default; the LDWEIGHTS
# double-buffer optimization hides weight-load time behind the previous
# matmul, which matters a lot for this kernel's many small matmuls.
_orig_run_command = _bass_utils.run_command


def _run_command_ldw(cmd, *a, **kw):
    cmd = [c.replace("--enable-ldw-opt=false", "--enable-ldw-opt=true")
           if isinstance(c, str) else c for c in cmd]
    return _orig_run_command(cmd, *a, **kw)


# NOTE: tried and reverted — walrus rejects explicit InstLdweights (which bass
# emits for 2-byte matmuls) when the opt is on: "InstLdweights is not
# compatible with LDW optimization".
if os.environ.get("ATTN_LDW_OPT", "0") == "1":
    _bass_utils.run_command = _run_command_ldw

F32 = mybir.dt.float32
F16 = mybir.dt.float16
EXP = mybir.ActivationFunctionType.Exp

S = 2048           # sequence length
DM = 768           # model dim
DH = 64            # head dim
NHL = 3            # local heads per core
FC = DM // 128     # 6 f-chunks (contraction for projections)
KC = S // 128      # 16 k-chunks
QB = 512           # q block width for score matmuls (one mm per k-chunk)
NQB = S // QB      # 4 q blocks
# k-chunk groups per exp op: [128, 2*512] PSUM tile = 2 banks
GROUPS = [(2 * i, 2) for i in range(8)]


def _build_kernel(zero_mask: bool) -> bass.Bass:
    nc = bacc.Bacc()

    x_d = nc.declare_dram_parameter("x", [S, DM], F32, isOutput=False)
    wq_d = nc.declare_dram_parameter("wq", [DM, 192], F32, isOutput=False)
    wk_d = nc.declare_dram_parameter("wk", [DM, 192], F32, isOutput=False)
    wv_d = nc.declare_dram_parameter("wv", [DM, 192], F32, isOutput=False)
    bq_d = nc.declare_dram_parameter("bq", [192], F32, isOutput=False)
    bk_d = nc.declare_dram_parameter("bk", [192], F32, isOutput=False)
    bv_d = nc.declare_dram_parameter("bv", [192], F32, isOutput=False)
    m_d = nc.declare_dram_parameter("mask", [S], F32, isOutput=False)
    out_d = nc.declare_dram_parameter("out", [S, 192], F32, isOutput=True)

    with tile.TileContext(nc) as tc:
        _attention(tc, x_d, (wq_d, wk_d, wv_d), (bq_d, bk_d, bv_d), m_d, out_d,
                   zero_mask)
    nc.compile()
    return nc


def _attention(tc, x_d, w_ds, b_ds, m_d, out_d, zero_mask):
    nc = tc.nc

    const = tc.alloc_tile_pool(name="const", bufs=1)
    xpool = tc.alloc_tile_pool(name="xpool", bufs=5)
    persist = tc.alloc_tile_pool(name="persist", bufs=1)
    probs_pool = tc.alloc_tile_pool(name="probs", bufs=50)
    small = tc.alloc_tile_pool(name="small", bufs=4)
    outp = tc.alloc_tile_pool(name="outp", bufs=1)
    ps = tc.alloc_tile_pool(name="ps", bufs=2, space="PSUM")

    # --- constants: mask, identity, weights, biases -------------------------
    # All small loads go through gpsimd (SWDGE) so the SP sequencer's serial
    # ~0.65us-per-DMA issue cost is spent exclusively on the large X loads.
    mask_t = const.tile([128, KC], F32)  # mask[128*i + p] at [p, i]
    nc.gpsimd.dma_start(out=mask_t, in_=m_d[:].rearrange("(i p) -> p i", p=128))
    expm = const.tile([128, KC], F32)    # exp(mask), per k position
    nc.scalar.activation(expm, mask_t, EXP)  # early: also triggers table load

    ident16 = const.tile([128, 128], F16)
    make_identity(nc, ident16)

    w16 = {}   # (t, f) -> [128, 192] fp16
    for t in range(3):
        for f in range(FC):
            w32 = small.tile([128, 192], F32, name=f"w32_{t}_{f}", tag="w32",
                             bufs=9)
            nc.gpsimd.dma_start(out=w32, in_=w_ds[t][128 * f:128 * (f + 1), :])
            wt = const.tile([128, 192], F16, name=f"w16_{t}_{f}")
            nc.vector.tensor_copy(out=wt, in_=w32)
            w16[(t, f)] = wt

    bias_pair = []
    bias_solo = []
    for t in range(3):
        bp = const.tile([128, 1], F32, name=f"bias_pair_{t}")
        nc.gpsimd.dma_start(out=bp, in_=b_ds[t][0:128].rearrange("(p o) -> p o", o=1))
        bias_pair.append(bp)
        bs = const.tile([64, 1], F32, name=f"bias_solo_{t}")
        nc.gpsimd.dma_start(out=bs, in_=b_ds[t][128:192].rearrange("(p o) -> p o", o=1))
        bias_solo.append(bs)

    # --- persistent projection outputs --------------------------------------
    # QT2/KT2: [128, 2048] fp16, rows 0:64 = head0, 64:128 = head1
    # QTs/KTs: [128, 2048] fp16, head2 duplicated into both partition halves
    XT = [persist.tile([128, S], F16, name=f"XT_{f}") for f in range(FC)]
    QT2 = persist.tile([128, S], F16)
    KT2 = persist.tile([128, S], F16)
    QTs = persist.tile([128, S], F16)
    KTs = persist.tile([128, S], F16)
    VT2 = persist.tile([128, S], F16)
    VTs = persist.tile([64, S], F16)
    # V[kc] layout: [V_h0(64) | e | V_h1(64) | e | V_h2(64) | e], e = exp(m_k)
    V = [persist.tile([128, 195], F16, name=f"V_{kc}") for kc in range(KC)]

    out_tiles = [outp.tile([128, 192], F32, name=f"o_{u}") for u in range(16)]
    out_written = [0] * 16

    def load_x_block(m):
        """DMA 4 q-tiles of X, cast fp16 on gpsimd, PE-transpose into XT."""
        x16s = []
        for j in range(4):
            qt = 4 * m + j
            xt32 = xpool.tile([128, DM], F32, name=f"x_{qt}", tag="x")
            nc.sync.dma_start(out=xt32, in_=x_d[128 * qt:128 * (qt + 1), :])
            x16 = xpool.tile([128, DM], F16, name=f"x16_{qt}", tag="x16")
            nc.vector.tensor_copy(out=x16, in_=xt32)
            x16s.append(x16)
        for f in range(FC):
            tp = ps.tile([128, 512], F16, name=f"xt_ps_{m}_{f}", tag="sm", bufs=4)
            for j in range(4):
                nc.tensor.transpose(
                    tp[:, 128 * j:128 * (j + 1)],
                    x16s[j][:, 128 * f:128 * (f + 1)],
                    ident16,
                )
            nc.vector.tensor_copy(out=XT[f][:, 512 * m:512 * (m + 1)], in_=tp)

    def proj_pair(t, dst_pair, m):
        cols = slice(512 * m, 512 * (m + 1))
        pp = ps.tile([128, 512], F32, name=f"proj_{t}_{m}_p", tag="sm", bufs=4)
        for f in range(FC):
            nc.tensor.matmul(pp, w16[(t, f)][:, 0:128], XT[f][:, cols],
                             start=(f == 0), stop=(f == FC - 1))
        nc.vector.tensor_scalar_add(out=dst_pair[:, cols], in0=pp,
                                    scalar1=bias_pair[t])

    def proj_solo2(ta, tb, dst_a, dst_b, m):
        """Two M=64 head-2 projections col-tiled into one PSUM bank: ta's
        output at partitions 0:64, tb's at 64:128, sharing the streamed X_T.
        Only ta's first matmul carries start=True (it clears the whole bank's
        has_written bits); tb's first is pinned after it and overwrites its
        cleared region."""
        cols = slice(512 * m, 512 * (m + 1))
        sp = ps.tile([128, 512], F32, name=f"proj_s_{m}", tag="sm", bufs=4)
        prev = None
        for f in range(FC):
            ma = nc.tensor.matmul(sp[0:64], w16[(ta, f)][:, 128:192],
                                  XT[f][:, cols],
                                  start=(f == 0), stop=(f == FC - 1))
            if prev is not None:
                tile.add_dep_helper(ma.ins, prev.ins, sync=False,
                                    reason="solo col-pair order")
            mb = nc.tensor.matmul(sp[64:128], w16[(tb, f)][:, 128:192],
                                  XT[f][:, cols],
                                  start=False, stop=(f == FC - 1))
            tile.add_dep_helper(mb.ins, ma.ins, sync=False,
                                reason="solo col-pair order")
            prev = mb
        nc.vector.tensor_scalar_add(out=dst_a[0:64, cols], in0=sp[0:64],
                                    scalar1=bias_solo[ta])
        nc.vector.tensor_scalar_add(out=dst_b[64:128, cols], in0=sp[64:128],
                                    scalar1=bias_solo[tb])
        # duplicate head2 into the other partition half for row tiling
        nc.sync.dma_start(out=dst_a[64:128, cols], in_=dst_a[0:64, cols])
        nc.sync.dma_start(out=dst_b[0:64, cols], in_=dst_b[64:128, cols])

    def proj_solo1(t, dst, m):
        cols = slice(512 * m, 512 * (m + 1))
        sp = ps.tile([128, 512], F32, name=f"proj_v_{m}", tag="sm", bufs=4)
        for f in range(FC):
            nc.tensor.matmul(sp[0:64], w16[(t, f)][:, 128:192], XT[f][:, cols],
                             start=(f == 0), stop=(f == FC - 1))
        nc.vector.tensor_scalar_add(out=dst[0:64, cols], in0=sp[0:64],
                                    scalar1=bias_solo[t])

    def build_v(kc):
        """Transpose V_T chunk back to [k, d], scale by exp(mask), add e col."""
        ks = slice(128 * kc, 128 * (kc + 1))
        vp = ps.tile([128, 192], F16, name=f"v_ps_{kc}", tag="sm", bufs=4)
        nc.tensor.transpose(vp[:, 0:128], VT2[:, ks], ident16)
        nc.tensor.transpose(vp[:, 128:192], VTs[:, ks], ident16[0:64, 0:64])
        ecol = bass.AP(tensor=V[kc].tensor, offset=V[kc].offset + 64,
                       ap=[V[kc].ap[0], [65, 3]])
        if zero_mask:
            for h in range(NHL):
                nc.vector.tensor_copy(
                    out=V[kc][:, 65 * h:65 * h + 64],
                    in_=vp[:, 64 * h:64 * h + 64])
            nc.gpsimd.memset(ecol, 1.0)
        else:
            sc = expm[:, kc:kc + 1]
            for h in range(NHL):
                nc.vector.tensor_scalar_mul(
                    out=V[kc][:, 65 * h:65 * h + 64],
                    in0=vp[:, 64 * h:64 * h + 64], scalar1=sc)
            esrc = bass.AP(tensor=expm.tensor, offset=expm.offset + kc,
                           ap=[expm.ap[0], [0, 3]])
            nc.vector.tensor_copy(out=ecol, in_=esrc)

    def scores_group(streams, g):
        """Row-tile-interleaved score matmuls + exp for two streams.

        streams: [(head, J, prow), (head, J, prow)] with prow 0 and 64.
        J is a 512-wide q block; one N=512 matmul per k-chunk per stream.
        Returns the two probs tiles."""
        k0, kn = GROUPS[g]
        scs = []
        for (head, J, prow) in streams:
            scs.append(ps.tile([128, QB * kn], F32,
                               name=f"sc_{head}_{J}_{g}", tag="sc", bufs=2))
        # N=256 half-matmuls, alternating the two streams' row groups: the PE
        # runs rows 0-63 and 64-127 concurrently (measured pair start skew
        # ~4ns), which doubles effective throughput for this K=64 contraction.
        prev_mm = None
        for j in range(kn):
            kc = k0 + j
            for h in range(2):
                for i, (head, J, prow) in enumerate(streams):
                    KT = KT2 if head < 2 else KTs
                    QT = QT2 if head < 2 else QTs
                    mm = nc.tensor.matmul(
                        scs[i][:, QB * j + 256 * h:QB * j + 256 * (h + 1)],
                        KT[prow:prow + 64, 128 * kc:128 * (kc + 1)],
                        QT[prow:prow + 64,
                           QB * J + 256 * h:QB * J + 256 * (h + 1)],
                        start=True, stop=True)
                    if i == 1 and prev_mm is not None:
                        tile.add_dep_helper(mm.ins, prev_mm.ins, sync=False,
                                            reason="score pair adjacency")
                    prev_mm = mm if i == 0 else None
        pts = []
        for i, (head, J, prow) in enumerate(streams):
            pt = probs_pool.tile([128, QB * kn], F16,
                                 name=f"pb_{head}_{J}_{g}", tag="probs")
            nc.scalar.activation(pt, scs[i], EXP, scale=0.125)
            pts.append(pt)
        return pts

    def ctx_chain(head, J, probs, s):
        """One q-sub-chunk's ctx accumulation + normalize + out.

        One PSUM tile (= one bank) per accumulation chain: start=True clears
        has_written for the whole bank, so chains must not share a bank."""
        cx = ps.tile([128, 65], F32, name=f"cx_{head}_{J}_{s}", tag="sm", bufs=4)
        for g, (k0, kn) in enumerate(GROUPS):
            for j in range(kn):
                kc = k0 + j
                nc.tensor.matmul(
                    cx,
                    probs[g][:, QB * j + 128 * s:QB * j + 128 * (s + 1)],
                    V[kc][:, 65 * head:65 * head + 65],
                    start=(kc == 0), stop=(kc == KC - 1))
        r = small.tile([128, 1], F32, name=f"r_{head}_{J}_{s}", tag="recip")
        nc.vector.reciprocal(r, cx[:, 64:65])
        u = 4 * J + s
        nc.vector.tensor_scalar_mul(
            out=out_tiles[u][:, 64 * head:64 * (head + 1)],
            in0=cx[:, 0:64], scalar1=r)
        out_written[u] += 1
        if out_written[u] == NHL:
            nc.sync.dma_start(out=out_d[128 * u:128 * (u + 1), :],
                              in_=out_tiles[u])

    # --- emission ------------------------------------------------------------
    # Score groups for ALL steps are spread across the projection m-blocks as
    # soon as their K/Q column blocks exist, so ScalarE's exp work overlaps the
    # whole projection phase.  A cap on un-consumed probs tiles bounds SBUF.
    all_steps = [
        ([(0, 0, 0), (1, 0, 64)], 0),
        ([(0, 1, 0), (1, 1, 64)], 1),
        ([(2, 0, 0), (2, 1, 64)], 1),
        ([(0, 2, 0), (1, 2, 64)], 2),
        ([(0, 3, 0), (1, 3, 64)], 3),
        ([(2, 2, 0), (2, 3, 64)], 3),
    ]
    units = [(si, g) for si in range(len(all_steps)) for g in range(len(GROUPS))]
    emitted = set()
    step_probs = {si: [[None] * len(GROUPS) for _ in range(2)]
                  for si in range(len(all_steps))}
    state = {"inflight": 0}
    CAP = 21  # max un-consumed score groups (2 probs tiles each)

    def emit_unit(si, g):
        streams, _ = all_steps[si]
        pts = scores_group(streams, g)
        step_probs[si][0][g] = pts[0]
        step_probs[si][1][g] = pts[1]
        emitted.add((si, g))
        state["inflight"] += 1

    def try_emit(q_m, k_m, budget, pair_only=False):
        for (si, g) in units:
            if budget <= 0 or state["inflight"] >= CAP:
                return
            if (si, g) in emitted:
                continue
            if pair_only and all_steps[si][0][0][0] == 2:
                continue
            k0, kn = GROUPS[g]
            if all_steps[si][1] <= q_m and (k0 + kn - 1) // 4 <= k_m:
                emit_unit(si, g)
                budget -= 1

    for m in range(4):
        load_x_block(m)
        proj_pair(1, KT2, m)   # K first: scores need all of K
        try_emit(m - 1, m - 1, 3)
        proj_pair(0, QT2, m)
        try_emit(m, m, 3, pair_only=True)
        proj_solo2(0, 1, QTs, KTs, m)
        try_emit(m, m, 3)
        proj_pair(2, VT2, m)
        try_emit(m, m, 3)
        proj_solo1(2, VTs, m)
        for kc in range(4 * m, 4 * m + 4):
            build_v(kc)
        try_emit(m, m, 3)

    # Steady state: remaining score groups interleaved with ctx chains of
    # completed steps, so the PE's ctx work overlaps ScalarE's exp work.
    pending = []
    for si, (streams, _) in enumerate(all_steps):
        for g in range(len(GROUPS)):
            if (si, g) not in emitted:
                if pending:
                    ctx_chain(*pending.pop(0))
                emit_unit(si, g)
        for s in range(4):
            for i in range(2):
                pending.append((streams[i][0], streams[i][1],
                                step_probs[si][i], s))
        state["inflight"] -= len(GROUPS)
    while pending:
        ctx_chain(*pending.pop(0))

    for p in (ps, outp, small, probs_pool, persist, xpool, const):
        p.release()


_NC_CACHE = {}


def _get_nc(zero_mask: bool):
    if zero_mask not in _NC_CACHE:
        _NC_CACHE[zero_mask] = _build_kernel(zero_mask)
    return _NC_CACHE[zero_mask]


def kernel(hidden_states, attention_mask, Wq, bq, Wk, bk, Wv, bv, **run_kw):
    hidden_states = np.asarray(hidden_states, dtype=np.float32)
    attention_mask = np.asarray(attention_mask, dtype=np.float32)
    Wq, Wk, Wv = (np.asarray(a, dtype=np.float32) for a in (Wq, Wk, Wv))
    bq, bk, bv = (np.asarray(a, dtype=np.float32) for a in (bq, bk, bv))

    zero_mask = bool(np.all(attention_mask == 0.0))
    nc = _get_nc(zero_mask)
    in_maps = []
    for c in range(8):
        b, g = c // 4, c % 4
        cols = slice(192 * g, 192 * (g + 1))
        in_maps.append({
            "x": np.ascontiguousarray(hidden_states[b]),
            "wq": np.ascontiguousarray(Wq[:, cols]),
            "wk": np.ascontiguousarray(Wk[:, cols]),
            "wv": np.ascontiguousarray(Wv[:, cols]),
            "bq": np.ascontiguousarray(bq[cols]),
            "bk": np.ascontiguousarray(bk[cols]),
            "bv": np.ascontiguousarray(bv[cols]),
            "mask": np.ascontiguousarray(
                np.broadcast_to(attention_mask[b, 0, 0], (S,))),
        })
    res = run_bass_kernel_spmd(nc, in_maps, list(range(8)), **run_kw)
    out = np.empty((2, S, DM), dtype=np.float32)
    for c in range(8):
        b, g = c // 4, c % 4
        out[b, :, 192 * g:192 * (g + 1)] = res.results[c]["out"]
    if run_kw:
        return out, res
    return out


# revision 56
# speedup vs baseline: 1.0158x; 1.0100x over previous
"""BERT self-attention Bass/Tile kernel for 8 Trainium2 NeuronCores.

Problem: hidden [2, 2048, 768], 12 heads x 64 dim, additive mask [2,1,1,2048].
Sharding: batch x head-group. Core c handles batch b = c // 4 and global heads
3*(c%4) .. 3*(c%4)+2 (columns 192*(c%4) .. +192 of Wq/Wk/Wv).  Each core
computes its 3 heads' full attention locally; outputs are concatenated on the
host (no cross-device communication).

Per-core pipeline (all in one TileContext):
  X [2048,768] --cast fp16, PE transpose--> X_T [768, 2048]
  Q_T/K_T/V_T = W.T @ X_T   (fp16 matmuls; pair of heads packed M=128 + solo)
  scores_T[k,q] = K_T.T @ Q_T  (K=64 contraction; two row-tiled streams
                                interleaved per k-chunk so the PE overlaps them)
  probs = exp(scores/8) via ScalarE (PSUM -> SBUF, fp16)
  V[k] layout [V_h0|e|V_h1|e|V_h2|e] where e = exp(mask_k) column
  ctx_aug[q, 65] = probs_T.T @ V_aug  (col 64 = softmax denominator)
  out[q, d] = ctx[:, :64] * (1 / ctx[:, 64])   -> DMA to DRAM

The additive mask is folded into V: exp(s + m_k) = exp(s) * exp(m_k), so both
the numerator and the denominator column of V_aug are pre-scaled by exp(m_k).
When the mask is all zeros (the common case) that scale is skipped and the
denominator column is just memset to 1.
"""

import os

import numpy as np

import concourse.bass as bass
import concourse.bass_utils as _bass_utils
import concourse.tile as tile
from concourse import bacc, mybir
from concourse.bass_utils import run_bass_kernel_spmd
from concourse.masks import make_identity

# walrus is invoked with --enable-ldw-opt=false by default; the LDWEIGHTS
# double-buffer optimization hides weight-load time behind the previous
# matmul, which matters a lot for this kernel's many small matmuls.
_orig_run_command = _bass_utils.run_command


def _run_command_ldw(cmd, *a, **kw):
    cmd = [c.replace("--enable-ldw-opt=false", "--enable-ldw-opt=true")
           if isinstance(c, str) else c for c in cmd]
    return _orig_run_command(cmd, *a, **kw)


# NOTE: tried and reverted — walrus rejects explicit InstLdweights (which bass
# emits for 2-byte matmuls) when the opt is on: "InstLdweights is not
# compatible with LDW optimization".
if os.environ.get("ATTN_LDW_OPT", "0") == "1":
    _bass_utils.run_command = _run_command_ldw

F32 = mybir.dt.float32
F16 = mybir.dt.float16
EXP = mybir.ActivationFunctionType.Exp

S = 2048           # sequence length
DM = 768           # model dim
DH = 64            # head dim
NHL = 3            # local heads per core
FC = DM // 128     # 6 f-chunks (contraction for projections)
KC = S // 128      # 16 k-chunks
QB = 512           # q block width for score matmuls (one mm per k-chunk)
NQB = S // QB      # 4 q blocks
# k-chunk groups per exp op: [128, 2*512] PSUM tile = 2 banks
GROUPS = [(2 * i, 2) for i in range(8)]


def _build_kernel(zero_mask: bool) -> bass.Bass:
    nc = bacc.Bacc()

    x_d = nc.declare_dram_parameter("x", [S, DM], F32, isOutput=False)
    wq_d = nc.declare_dram_parameter("wq", [DM, 192], F32, isOutput=False)
    wk_d = nc.declare_dram_parameter("wk", [DM, 192], F32, isOutput=False)
    wv_d = nc.declare_dram_parameter("wv", [DM, 192], F32, isOutput=False)
    bq_d = nc.declare_dram_parameter("bq", [192], F32, isOutput=False)
    bk_d = nc.declare_dram_parameter("bk", [192], F32, isOutput=False)
    bv_d = nc.declare_dram_parameter("bv", [192], F32, isOutput=False)
    m_d = nc.declare_dram_parameter("mask", [S], F32, isOutput=False)
    out_d = nc.declare_dram_parameter("out", [S, 192], F32, isOutput=True)

    with tile.TileContext(nc) as tc:
        _attention(tc, x_d, (wq_d, wk_d, wv_d), (bq_d, bk_d, bv_d), m_d, out_d,
                   zero_mask)
    nc.compile()
    return nc


def _attention(tc, x_d, w_ds, b_ds, m_d, out_d, zero_mask):
    nc = tc.nc

    const = tc.alloc_tile_pool(name="const", bufs=1)
    xpool = tc.alloc_tile_pool(name="xpool", bufs=5)
    persist = tc.alloc_tile_pool(name="persist", bufs=1)
    probs_pool = tc.alloc_tile_pool(name="probs", bufs=50)
    small = tc.alloc_tile_pool(name="small", bufs=4)
    outp = tc.alloc_tile_pool(name="outp", bufs=1)
    ps = tc.alloc_tile_pool(name="ps", bufs=2, space="PSUM")

    # --- constants: mask, identity, weights, biases -------------------------
    # All small loads go through gpsimd (SWDGE) so the SP sequencer's serial
    # ~0.65us-per-DMA issue cost is spent exclusively on the large X loads.
    mask_t = const.tile([128, KC], F32)  # mask[128*i + p] at [p, i]
    nc.gpsimd.dma_start(out=mask_t, in_=m_d[:].rearrange("(i p) -> p i", p=128))
    expm = const.tile([128, KC], F32)    # exp(mask), per k position
    nc.scalar.activation(expm, mask_t, EXP)  # early: also triggers table load

    ident16 = const.tile([128, 128], F16)
    make_identity(nc, ident16)

    w16 = {}   # (t, f) -> [128, 192] fp16
    for t in range(3):
        for f in range(FC):
            w32 = small.tile([128, 192], F32, name=f"w32_{t}_{f}", tag="w32",
                             bufs=9)
            nc.gpsimd.dma_start(out=w32, in_=w_ds[t][128 * f:128 * (f + 1), :])
            wt = const.tile([128, 192], F16, name=f"w16_{t}_{f}")
            nc.vector.tensor_copy(out=wt, in_=w32)
            w16[(t, f)] = wt

    bias_pair = []
    bias_solo = []
    for t in range(3):
        bp = const.tile([128, 1], F32, name=f"bias_pair_{t}")
        nc.gpsimd.dma_start(out=bp, in_=b_ds[t][0:128].rearrange("(p o) -> p o", o=1))
        bias_pair.append(bp)
        bs = const.tile([64, 1], F32, name=f"bias_solo_{t}")
        nc.gpsimd.dma_start(out=bs, in_=b_ds[t][128:192].rearrange("(p o) -> p o", o=1))
        bias_solo.append(bs)

    # --- persistent projection outputs --------------------------------------
    # QT2/KT2: [128, 2048] fp16, rows 0:64 = head0, 64:128 = head1
    # QTs/KTs: [128, 2048] fp16, head2 duplicated into both partition halves
    XT = [persist.tile([128, S], F16, name=f"XT_{f}") for f in range(FC)]
    QT2 = persist.tile([128, S], F16)
    KT2 = persist.tile([128, S], F16)
    QTs = persist.tile([128, S], F16)
    KTs = persist.tile([128, S], F16)
    VT2 = persist.tile([128, S], F16)
    VTs = persist.tile([64, S], F16)
    # V[kc] layout: [V_h0(64) | e | V_h1(64) | e | V_h2(64) | e], e = exp(m_k)
    V = [persist.tile([128, 195], F16, name=f"V_{kc}") for kc in range(KC)]

    out_tiles = [outp.tile([128, 192], F32, name=f"o_{u}") for u in range(16)]
    out_written = [0] * 16

    def load_x_block(m):
        """DMA 4 q-tiles of X, cast fp16 on DVE, PE-transpose into XT."""
        x16s = []
        for j in range(4):
            qt = 4 * m + j
            xt32 = xpool.tile([128, DM], F32, name=f"x_{qt}", tag="x")
            nc.sync.dma_start(out=xt32, in_=x_d[128 * qt:128 * (qt + 1), :])
            x16 = xpool.tile([128, DM], F16, name=f"x16_{qt}", tag="x16")
            nc.vector.tensor_copy(out=x16, in_=xt32)
            x16s.append(x16)
        for f in range(FC):
            tp = ps.tile([128, 512], F16, name=f"xt_ps_{m}_{f}", tag="sm", bufs=4)
            for j in range(4):
                nc.tensor.transpose(
                    tp[:, 128 * j:128 * (j + 1)],
                    x16s[j][:, 128 * f:128 * (f + 1)],
                    ident16,
                )
            nc.vector.tensor_copy(out=XT[f][:, 512 * m:512 * (m + 1)], in_=tp)

    def proj_pair(t, dst_pair, m):
        cols = slice(512 * m, 512 * (m + 1))
        pp = ps.tile([128, 512], F32, name=f"proj_{t}_{m}_p", tag="sm", bufs=4)
        for f in range(FC):
            nc.tensor.matmul(pp, w16[(t, f)][:, 0:128], XT[f][:, cols],
                             start=(f == 0), stop=(f == FC - 1))
        nc.vector.tensor_scalar_add(out=dst_pair[:, cols], in0=pp,
                                    scalar1=bias_pair[t])

    def proj_solo2(ta, tb, dst_a, dst_b, m):
        """Two M=64 head-2 projections col-tiled into one PSUM bank: ta's
        output at partitions 0:64, tb's at 64:128, sharing the streamed X_T.
        Only ta's first matmul carries start=True (it clears the whole bank's
        has_written bits); tb's first is pinned after it and overwrites its
        cleared region."""
        cols = slice(512 * m, 512 * (m + 1))
        sp = ps.tile([128, 512], F32, name=f"proj_s_{m}", tag="sm", bufs=4)
        prev = None
        for f in range(FC):
            ma = nc.tensor.matmul(sp[0:64], w16[(ta, f)][:, 128:192],
                                  XT[f][:, cols],
                                  start=(f == 0), stop=(f == FC - 1))
            if prev is not None:
                tile.add_dep_helper(ma.ins, prev.ins, sync=False,
                                    reason="solo col-pair order")
            mb = nc.tensor.matmul(sp[64:128], w16[(tb, f)][:, 128:192],
                                  XT[f][:, cols],
                                  start=False, stop=(f == FC - 1))
            tile.add_dep_helper(mb.ins, ma.ins, sync=False,
                                reason="solo col-pair order")
            prev = mb
        nc.vector.tensor_scalar_add(out=dst_a[0:64, cols], in0=sp[0:64],
                                    scalar1=bias_solo[ta])
        nc.vector.tensor_scalar_add(out=dst_b[64:128, cols], in0=sp[64:128],
                                    scalar1=bias_solo[tb])
        # duplicate head2 into the other partition half for row tiling
        nc.sync.dma_start(out=dst_a[64:128, cols], in_=dst_a[0:64, cols])
        nc.sync.dma_start(out=dst_b[0:64, cols], in_=dst_b[64:128, cols])

    def proj_solo1(t, dst, m):
        cols = slice(512 * m, 512 * (m + 1))
        sp = ps.tile([128, 512], F32, name=f"proj_v_{m}", tag="sm", bufs=4)
        for f in range(FC):
            nc.tensor.matmul(sp[0:64], w16[(t, f)][:, 128:192], XT[f][:, cols],
                             start=(f == 0), stop=(f == FC - 1))
        nc.vector.tensor_scalar_add(out=dst[0:64, cols], in0=sp[0:64],
                                    scalar1=bias_solo[t])

    def build_v(kc):
        """Transpose V_T chunk back to [k, d], scale by exp(mask), add e col."""
        ks = slice(128 * kc, 128 * (kc + 1))
        vp = ps.tile([128, 192], F16, name=f"v_ps_{kc}", tag="sm", bufs=4)
        nc.tensor.transpose(vp[:, 0:128], VT2[:, ks], ident16)
        nc.tensor.transpose(vp[:, 128:192], VTs[:, ks], ident16[0:64, 0:64])
        ecol = bass.AP(tensor=V[kc].tensor, offset=V[kc].offset + 64,
                       ap=[V[kc].ap[0], [65, 3]])
        if zero_mask:
            for h in range(NHL):
                nc.vector.tensor_copy(
                    out=V[kc][:, 65 * h:65 * h + 64],
                    in_=vp[:, 64 * h:64 * h + 64])
            nc.gpsimd.memset(ecol, 1.0)
        else:
            sc = expm[:, kc:kc + 1]
            for h in range(NHL):
                nc.vector.tensor_scalar_mul(
                    out=V[kc][:, 65 * h:65 * h + 64],
                    in0=vp[:, 64 * h:64 * h + 64], scalar1=sc)
            esrc = bass.AP(tensor=expm.tensor, offset=expm.offset + kc,
                           ap=[expm.ap[0], [0, 3]])
            nc.vector.tensor_copy(out=ecol, in_=esrc)

    def scores_group(streams, g):
        """Row-tile-interleaved score matmuls + exp for two streams.

        streams: [(head, J, prow), (head, J, prow)] with prow 0 and 64.
        J is a 512-wide q block; one N=512 matmul per k-chunk per stream.
        Returns the two probs tiles."""
        k0, kn = GROUPS[g]
        scs = []
        for (head, J, prow) in streams:
            scs.append(ps.tile([128, QB * kn], F32,
                               name=f"sc_{head}_{J}_{g}", tag="sc", bufs=2))
        # N=256 half-matmuls, alternating the two streams' row groups: the PE
        # runs rows 0-63 and 64-127 concurrently (measured pair start skew
        # ~4ns), which doubles effective throughput for this K=64 contraction.
        prev_mm = None
        for j in range(kn):
            kc = k0 + j
            for h in range(2):
                for i, (head, J, prow) in enumerate(streams):
                    KT = KT2 if head < 2 else KTs
                    QT = QT2 if head < 2 else QTs
                    mm = nc.tensor.matmul(
                        scs[i][:, QB * j + 256 * h:QB * j + 256 * (h + 1)],
                        KT[prow:prow + 64, 128 * kc:128 * (kc + 1)],
                        QT[prow:prow + 64,
                           QB * J + 256 * h:QB * J + 256 * (h + 1)],
                        start=True, stop=True)
                    if i == 1 and prev_mm is not None:
                        tile.add_dep_helper(mm.ins, prev_mm.ins, sync=False,
                                            reason="score pair adjacency")
                    prev_mm = mm if i == 0 else None
        pts = []
        for i, (head, J, prow) in enumerate(streams):
            pt = probs_pool.tile([128, QB * kn], F16,
                                 name=f"pb_{head}_{J}_{g}", tag="probs")
            nc.scalar.activation(pt, scs[i], EXP, scale=0.125)
            pts.append(pt)
        return pts

    def ctx_chain(head, J, probs, s):
        """One q-sub-chunk's ctx accumulation + normalize + out.

        One PSUM tile (= one bank) per accumulation chain: start=True clears
        has_written for the whole bank, so chains must not share a bank."""
        cx = ps.tile([128, 65], F32, name=f"cx_{head}_{J}_{s}", tag="sm", bufs=4)
        for g, (k0, kn) in enumerate(GROUPS):
            for j in range(kn):
                kc = k0 + j
                nc.tensor.matmul(
                    cx,
                    probs[g][:, QB * j + 128 * s:QB * j + 128 * (s + 1)],
                    V[kc][:, 65 * head:65 * head + 65],
                    start=(kc == 0), stop=(kc == KC - 1))
        r = small.tile([128, 1], F32, name=f"r_{head}_{J}_{s}", tag="recip")
        nc.vector.reciprocal(r, cx[:, 64:65])
        u = 4 * J + s
        nc.vector.tensor_scalar_mul(
            out=out_tiles[u][:, 64 * head:64 * (head + 1)],
            in0=cx[:, 0:64], scalar1=r)
        out_written[u] += 1
        if out_written[u] == NHL:
            nc.sync.dma_start(out=out_d[128 * u:128 * (u + 1), :],
                              in_=out_tiles[u])

    # --- emission ------------------------------------------------------------
    # Score groups for ALL steps are spread across the projection m-blocks as
    # soon as their K/Q column blocks exist, so ScalarE's exp work overlaps the
    # whole projection phase.  A cap on un-consumed probs tiles bounds SBUF.
    all_steps = [
        ([(0, 0, 0), (1, 0, 64)], 0),
        ([(0, 1, 0), (1, 1, 64)], 1),
        ([(2, 0, 0), (2, 1, 64)], 1),
        ([(0, 2, 0), (1, 2, 64)], 2),
        ([(0, 3, 0), (1, 3, 64)], 3),
        ([(2, 2, 0), (2, 3, 64)], 3),
    ]
    units = [(si, g) for si in range(len(all_steps)) for g in range(len(GROUPS))]
    emitted = set()
    step_probs = {si: [[None] * len(GROUPS) for _ in range(2)]
                  for si in range(len(all_steps))}
    state = {"inflight": 0}
    CAP = 21  # max un-consumed score groups (2 probs tiles each)

    def emit_unit(si, g):
        streams, _ = all_steps[si]
        pts = scores_group(streams, g)
        step_probs[si][0][g] = pts[0]
        step_probs[si][1][g] = pts[1]
        emitted.add((si, g))
        state["inflight"] += 1

    def try_emit(q_m, k_m, budget, pair_only=False):
        for (si, g) in units:
            if budget <= 0 or state["inflight"] >= CAP:
                return
            if (si, g) in emitted:
                continue
            if pair_only and all_steps[si][0][0][0] == 2:
                continue
            k0, kn = GROUPS[g]
            if all_steps[si][1] <= q_m and (k0 + kn - 1) // 4 <= k_m:
                emit_unit(si, g)
                budget -= 1

    for m in range(4):
        load_x_block(m)
        proj_pair(1, KT2, m)   # K first: scores need all of K
        try_emit(m - 1, m - 1, 3)
        proj_pair(0, QT2, m)
        try_emit(m, m, 3, pair_only=True)
        proj_solo2(0, 1, QTs, KTs, m)
        try_emit(m, m, 3)
        proj_pair(2, VT2, m)
        try_emit(m, m, 3)
        proj_solo1(2, VTs, m)
        for kc in range(4 * m, 4 * m + 4):
            build_v(kc)
        try_emit(m, m, 3)

    # Steady state: remaining score groups interleaved with ctx chains of
    # completed steps, so the PE's ctx work overlaps ScalarE's exp work.
    pending = []
    for si, (streams, _) in enumerate(all_steps):
        for g in range(len(GROUPS)):
            if (si, g) not in emitted:
                if pending:
                    ctx_chain(*pending.pop(0))
                emit_unit(si, g)
        for s in range(4):
            for i in range(2):
                pending.append((streams[i][0], streams[i][1],
                                step_probs[si][i], s))
        state["inflight"] -= len(GROUPS)
    while pending:
        ctx_chain(*pending.pop(0))

    for p in (ps, outp, small, probs_pool, persist, xpool, const):
        p.release()


_NC_CACHE = {}


def _get_nc(zero_mask: bool):
    if zero_mask not in _NC_CACHE:
        _NC_CACHE[zero_mask] = _build_kernel(zero_mask)
    return _NC_CACHE[zero_mask]


def kernel(hidden_states, attention_mask, Wq, bq, Wk, bk, Wv, bv, **run_kw):
    hidden_states = np.asarray(hidden_states, dtype=np.float32)
    attention_mask = np.asarray(attention_mask, dtype=np.float32)
    Wq, Wk, Wv = (np.asarray(a, dtype=np.float32) for a in (Wq, Wk, Wv))
    bq, bk, bv = (np.asarray(a, dtype=np.float32) for a in (bq, bk, bv))

    zero_mask = bool(np.all(attention_mask == 0.0))
    nc = _get_nc(zero_mask)
    in_maps = []
    for c in range(8):
        b, g = c // 4, c % 4
        cols = slice(192 * g, 192 * (g + 1))
        in_maps.append({
            "x": np.ascontiguousarray(hidden_states[b]),
            "wq": np.ascontiguousarray(Wq[:, cols]),
            "wk": np.ascontiguousarray(Wk[:, cols]),
            "wv": np.ascontiguousarray(Wv[:, cols]),
            "bq": np.ascontiguousarray(bq[cols]),
            "bk": np.ascontiguousarray(bk[cols]),
            "bv": np.ascontiguousarray(bv[cols]),
            "mask": np.ascontiguousarray(
                np.broadcast_to(attention_mask[b, 0, 0], (S,))),
        })
    res = run_bass_kernel_spmd(nc, in_maps, list(range(8)), **run_kw)
    out = np.empty((2, S, DM), dtype=np.float32)
    for c in range(8):
        b, g = c // 4, c % 4
        out[b, :, 192 * g:192 * (g + 1)] = res.results[c]["out"]
    if run_kw:
        return out, res
    return out
